# revision 5
# baseline (speedup 1.0000x reference)
"""DeepHamCritic (3x GCNConv + dense head) on 8 trn2 NeuronCores.

Strategy:
  - Host: build dense normalized adjacency A [1000,1000] from edge_index
    (self-loops + deg^-1/2 symmetric norm), pad nodes 1000 -> 1024.
  - GCN layers sharded by destination node (128 padded nodes per core):
    aggregation = dense matmul against the local A^T column slice,
    AllGather of node features between layers.
  - Dense head: Wd1 [512000,256] row-sharded (64Ki rows/core, fp16),
    streamed through SBUF slabs into a PE matvec accumulated in PSUM;
    partial [256] results AllGather'ed and summed on-chip; the tiny
    Wd2/Wd3/Wd4 layers are computed replicated on every core.
"""

import numpy as np

N_CORES = 8
N = 1000          # real nodes
P = 1024          # padded nodes
S = P // N_CORES  # nodes per core = 128
F = 128           # input features
D = 512           # GCN hidden
H = 256           # dense hidden
KCH = P * D // N_CORES // 128   # 512 f-chunks of 128 per core
SLAB_CH = 32                    # chunks per DMA slab
N_SLAB = KCH // SLAB_CH         # 16 slabs
SLAB_W = SLAB_CH * H            # 8192 fp16 cols per slab

_NC = None


def _build_nc():
    import concourse.bacc as bacc
    import concourse.mybir as mybir
    import concourse.tile as tile

    f32 = mybir.dt.float32
    f16 = mybir.dt.float16
    RG = [list(range(N_CORES))]

    nc = bacc.Bacc("TRN2", target_bir_lowering=False, debug=False,
                   num_devices=N_CORES)

    # ---- I/O ----
    xk = nc.dram_tensor("xk", [P, F], f32, kind="ExternalInput")
    ats = nc.dram_tensor("ats", [P, S], f32, kind="ExternalInput")
    w1 = nc.dram_tensor("w1", [F, D], f32, kind="ExternalInput")
    w2 = nc.dram_tensor("w2", [D, D], f32, kind="ExternalInput")
    w3 = nc.dram_tensor("w3", [D, D], f32, kind="ExternalInput")
    b1bc = nc.dram_tensor("b1bc", [128, D], f32, kind="ExternalInput")
    b2bc = nc.dram_tensor("b2bc", [128, D], f32, kind="ExternalInput")
    b3bc = nc.dram_tensor("b3bc", [128, D], f32, kind="ExternalInput")
    ident = nc.dram_tensor("ident", [128, 128], f32, kind="ExternalInput")
    wd1s = nc.dram_tensor("wd1s", [128, KCH * H], f16, kind="ExternalInput")
    wd2 = nc.dram_tensor("wd2", [H, H], f32, kind="ExternalInput")
    wd3 = nc.dram_tensor("wd3", [H, H], f32, kind="ExternalInput")
    wd4 = nc.dram_tensor("wd4", [H, 1], f32, kind="ExternalInput")
    bd1c = nc.dram_tensor("bd1c", [128, 2], f32, kind="ExternalInput")
    bd2c = nc.dram_tensor("bd2c", [128, 2], f32, kind="ExternalInput")
    bd3c = nc.dram_tensor("bd3c", [128, 2], f32, kind="ExternalInput")
    bd4 = nc.dram_tensor("bd4", [1, 1], f32, kind="ExternalInput")
    out = nc.dram_tensor("out", [1, 1], f32, kind="ExternalOutput")

    Tanh = mybir.ActivationFunctionType.Tanh
    Bypass = mybir.AluOpType.bypass

    with tile.TileContext(nc) as tc:
        with (
            tc.tile_pool(name="wslab", bufs=7) as wpool,
            tc.tile_pool(name="const", bufs=1) as cp,
            tc.tile_pool(name="hfull", bufs=8) as hp,
            tc.tile_pool(name="work", bufs=2) as wk,
            tc.tile_pool(name="psum", bufs=2, space="PSUM") as pp,
            tc.tile_pool(name="psacc", bufs=1, space="PSUM") as pacc,
            tc.tile_pool(name="dram", bufs=1, space="DRAM") as dp,
        ):
            # ---- load constants ----
            xk_t = []
            ats_t = []
            for q in range(8):
                t = cp.tile([128, F], f32, tag=f"xk{q}")
                nc.sync.dma_start(t[:], xk[q * 128:(q + 1) * 128, :])
                xk_t.append(t)
                a = cp.tile([128, S], f32, tag=f"ats{q}")
                nc.sync.dma_start(a[:], ats[q * 128:(q + 1) * 128, :])
                ats_t.append(a)
            w1_t = cp.tile([F, D], f32, tag="w1")
            nc.sync.dma_start(w1_t[:], w1[:])
            w2_t = []
            w3_t = []
            for m in range(4):
                t2 = cp.tile([128, D], f32, tag=f"w2{m}")
                nc.sync.dma_start(t2[:], w2[m * 128:(m + 1) * 128, :])
                w2_t.append(t2)
                t3 = cp.tile([128, D], f32, tag=f"w3{m}")
                nc.sync.dma_start(t3[:], w3[m * 128:(m + 1) * 128, :])
                w3_t.append(t3)
            b_t = []
            for nm, hndl in (("b1", b1bc), ("b2", b2bc), ("b3", b3bc)):
                t = cp.tile([128, D], f32, tag=nm)
                nc.sync.dma_start(t[:], hndl[:])
                b_t.append(t)
            id_t = cp.tile([128, 128], f32, tag="ident")
            nc.sync.dma_start(id_t[:], ident[:])
            wd2_t = []
            wd3_t = []
            for k in range(2):
                t2 = cp.tile([128, H], f32, tag=f"wd2{k}")
                nc.sync.dma_start(t2[:], wd2[k * 128:(k + 1) * 128, :])
                wd2_t.append(t2)
                t3 = cp.tile([128, H], f32, tag=f"wd3{k}")
                nc.sync.dma_start(t3[:], wd3[k * 128:(k + 1) * 128, :])
                wd3_t.append(t3)
            wd4_t = []
            for k in range(2):
                t4 = cp.tile([128, 1], f32, tag=f"wd4{k}")
                nc.sync.dma_start(t4[:], wd4[k * 128:(k + 1) * 128, :])
                wd4_t.append(t4)
            bd_t = []
            for nm, hndl in (("bd1", bd1c), ("bd2", bd2c), ("bd3", bd3c)):
                t = cp.tile([128, 2], f32, tag=nm)
                nc.sync.dma_start(t[:], hndl[:])
                bd_t.append(t)
            bd4_t = cp.tile([1, 1], f32, tag="bd4")
            nc.sync.dma_start(bd4_t[:], bd4[:])
            ones8 = cp.tile([8, 1], f32, tag="ones8")
            nc.vector.memset(ones8[:], 1.0)

            def leaky(dst_ap, ps_ap, bias_ap, mtag):
                t0 = wk.tile([128, 1], f32, tag=f"lk0{mtag}")
                nc.vector.tensor_add(t0[:], ps_ap, bias_ap)
                t1 = wk.tile([128, 1], f32, tag=f"lk1{mtag}")
                nc.vector.tensor_scalar_mul(t1[:], t0[:], 0.1)
                nc.vector.tensor_max(dst_ap, t0[:], t1[:])

            # ================= GCN =================
            # ---- layer 1 (local 128 dst nodes) ----
            ps_a1 = pp.tile([128, S], f32, tag="ps_sm")
            for k in range(8):
                nc.tensor.matmul(ps_a1[:], xk_t[k][:], ats_t[k][:],
                                 start=(k == 0), stop=(k == 7))
            a1 = wk.tile([128, S], f32, tag="a1")
            nc.vector.tensor_copy(a1[:], ps_a1[:])
            ps_h1 = pp.tile([128, D], f32, tag="ps_h")
            nc.tensor.matmul(ps_h1[:], a1[:], w1_t[:], start=True, stop=True)
            hb1 = wk.tile([128, D], f32, tag="hb")
            nc.vector.tensor_add(hb1[:], ps_h1[:], b_t[0][:])
            hs1 = wk.tile([128, D], f32, tag="hs")
            nc.scalar.activation(hs1[:], hb1[:], Tanh)

            # AllGather h1
            cc1i = dp.tile([128, D], f32, tag="cc1i")
            nc.sync.dma_start(cc1i[:], hs1[:])
            cc1o = dp.tile([P, D], f32, tag="cc1o", addr_space="Shared")
            nc.gpsimd.collective_compute(
                "AllGather", Bypass, replica_groups=RG,
                ins=[cc1i.opt()], outs=[cc1o.opt()])
            h1_t = []
            for q in range(8):
                t = hp.tile([128, D], f32, tag="hfull")
                nc.sync.dma_start(t[:], cc1o[q * 128:(q + 1) * 128, :])
                h1_t.append(t)

            def gcn_layer(h_in_t, w_chunks, b_tile, lname):
                # agg^T slices: [feat_chunk m partitions, S nodes]
                a2 = wk.tile([128, 4 * S], f32, tag=f"agg{lname}")
                for m in range(4):
                    ps = pp.tile([128, S], f32, tag="ps_sm")
                    for k in range(8):
                        nc.tensor.matmul(
                            ps[:], h_in_t[k][:, m * 128:(m + 1) * 128],
                            ats_t[k][:], start=(k == 0), stop=(k == 7))
                    nc.vector.tensor_copy(a2[:, m * S:(m + 1) * S], ps[:])
                ps_h = pp.tile([128, D], f32, tag="ps_h")
                for m in range(4):
                    nc.tensor.matmul(ps_h[:], a2[:, m * S:(m + 1) * S],
                                     w_chunks[m][:],
                                     start=(m == 0), stop=(m == 3))
                hb = wk.tile([128, D], f32, tag="hb")
                nc.vector.tensor_add(hb[:], ps_h[:], b_tile[:])
                hs = wk.tile([128, D], f32, tag="hs")
                nc.scalar.activation(hs[:], hb[:], Tanh)
                return hs

            # ---- layer 2 ----
            hs2 = gcn_layer(h1_t, w2_t, b_t[1], "l2")
            cc2i = dp.tile([128, D], f32, tag="cc2i")
            nc.sync.dma_start(cc2i[:], hs2[:])
            cc2o = dp.tile([P, D], f32, tag="cc2o", addr_space="Shared")
            nc.gpsimd.collective_compute(
                "AllGather", Bypass, replica_groups=RG,
                ins=[cc2i.opt()], outs=[cc2o.opt()])
            h2_t = []
            for q in range(8):
                t = hp.tile([128, D], f32, tag="hfull")
                nc.sync.dma_start(t[:], cc2o[q * 128:(q + 1) * 128, :])
                h2_t.append(t)

            # ---- layer 3 (local slice only; no gather) ----
            hs3 = gcn_layer(h2_t, w3_t, b_t[2], "l3")

            # ---- transpose local h3 [S,D] -> 4x [128, S] fp16 ----
            h3T = []
            for j in range(4):
                pst = pp.tile([128, S], f32, tag="ps_sm")
                nc.tensor.transpose(pst[:], hs3[:, j * 128:(j + 1) * 128],
                                    id_t[:])
                t16 = wk.tile([128, S], f16, tag=f"h3T{j}")
                nc.vector.tensor_copy(t16[:], pst[:])
                h3T.append(t16)

            # ================= dense head =================
            ps_y1 = pacc.tile([1, H], f32, tag="ps_y1")
            for g in range(N_SLAB):
                slab = wpool.tile([128, SLAB_W], f16, tag="slab")
                nc.sync.dma_start(slab[:], wd1s[:, g * SLAB_W:(g + 1) * SLAB_W])
                for t in range(SLAB_CH):
                    c = g * SLAB_CH + t
                    i, j = c // 4, c % 4
                    nc.tensor.matmul(
                        ps_y1[:], h3T[j][:, i:i + 1],
                        slab[:, t * H:(t + 1) * H],
                        start=(c == 0), stop=(c == KCH - 1))
            y1p = wk.tile([1, H], f32, tag="y1p")
            nc.vector.tensor_copy(y1p[:], ps_y1[:])

            # AllGather partials -> [8, H], then sum over partition dim
            ccyi = dp.tile([1, H], f32, tag="ccyi")
            nc.sync.dma_start(ccyi[:], y1p[:])
            ccyo = dp.tile([8, H], f32, tag="ccyo", addr_space="Shared")
            nc.gpsimd.collective_compute(
                "AllGather", Bypass, replica_groups=RG,
                ins=[ccyi.opt()], outs=[ccyo.opt()])
            y1g = wk.tile([8, H], f32, tag="y1g")
            nc.sync.dma_start(y1g[:], ccyo[:])

            y1c = wk.tile([128, 2], f32, tag="y1c")
            for m in range(2):
                ps = pp.tile([128, 1], f32, tag="ps_small")
                nc.tensor.matmul(ps[:], y1g[:, m * 128:(m + 1) * 128],
                                 ones8[:], start=True, stop=True)
                leaky(y1c[:, m:m + 1], ps[:], bd_t[0][:, m:m + 1], f"y1{m}")

            def dense(y_in, w_chunks, bias, oname):
                y_out = wk.tile([128, 2], f32, tag=oname)
                for m in range(2):
                    ps = pp.tile([128, 1], f32, tag="ps_small")
                    for k in range(2):
                        nc.tensor.matmul(
                            ps[:], w_chunks[k][:, m * 128:(m + 1) * 128],
                            y_in[:, k:k + 1], start=(k == 0), stop=(k == 1))
                    leaky(y_out[:, m:m + 1], ps[:], bias[:, m:m + 1],
                          f"{oname}{m}")
                return y_out

            y2c = dense(y1c, wd2_t, bd_t[1], "y2c")
            y3c = dense(y2c, wd3_t, bd_t[2], "y3c")

            ps_o = pp.tile([1, 1], f32, tag="ps_small")
            for k in range(2):
                nc.tensor.matmul(ps_o[:], wd4_t[k][:],
                                 y3c[:, k:k + 1], start=(k == 0), stop=(k == 1))
            out_sb = wk.tile([1, 1], f32, tag="out_sb")
            nc.vector.tensor_add(out_sb[:], ps_o[:], bd4_t[:])
            nc.sync.dma_start(out[:], out_sb[:])

    nc.compile()
    return nc


def _get_nc():
    global _NC
    if _NC is None:
        _NC = _build_nc()
    return _NC


def make_in_maps(inputs):
    """Host-side sharding / preprocessing. Returns per-core input dicts."""
    x = np.ascontiguousarray(np.asarray(inputs["x"], dtype=np.float32))
    ei = np.asarray(inputs["edge_index"])
    W1 = np.asarray(inputs["W1"], np.float32)
    W2 = np.asarray(inputs["W2"], np.float32)
    W3 = np.asarray(inputs["W3"], np.float32)
    b1 = np.asarray(inputs["b1"], np.float32)
    b2 = np.asarray(inputs["b2"], np.float32)
    b3 = np.asarray(inputs["b3"], np.float32)
    Wd1 = np.asarray(inputs["Wd1"], np.float32)
    Wd2 = np.asarray(inputs["Wd2"], np.float32)
    Wd3 = np.asarray(inputs["Wd3"], np.float32)
    Wd4 = np.asarray(inputs["Wd4"], np.float32)
    bd1 = np.asarray(inputs["bd1"], np.float32)
    bd2 = np.asarray(inputs["bd2"], np.float32)
    bd3 = np.asarray(inputs["bd3"], np.float32)
    bd4 = np.asarray(inputs["bd4"], np.float32)

    # normalized adjacency with self loops (GCNConv)
    src = ei[0].astype(np.int64)
    dst = ei[1].astype(np.int64)
    loop = np.arange(N, dtype=np.int64)
    s_all = np.concatenate([src, loop])
    d_all = np.concatenate([dst, loop])
    deg = np.bincount(d_all, minlength=N).astype(np.float32)
    dinv = np.where(deg > 0, 1.0 / np.sqrt(deg), 0.0).astype(np.float32)
    wnorm = dinv[s_all] * dinv[d_all]
    A = np.zeros((N, N), np.float32)
    np.add.at(A, (d_all, s_all), wnorm)
    AT = np.zeros((P, P), np.float32)
    AT[:N, :N] = A.T

    xk = np.zeros((P, F), np.float32)
    xk[:N] = x

    Wd1p = np.zeros((P * D, H), np.float16)
    Wd1p[:N * D] = Wd1.astype(np.float16)

    bb = lambda b: np.ascontiguousarray(np.broadcast_to(b[None, :], (128, b.shape[0])))
    bdc = lambda b: np.ascontiguousarray(b.reshape(2, 128).T)

    common = {
        "xk": xk,
        "w1": W1, "w2": W2, "w3": W3,
        "b1bc": bb(b1), "b2bc": bb(b2), "b3bc": bb(b3),
        "ident": np.eye(128, dtype=np.float32),
        "wd2": Wd2, "wd3": Wd3, "wd4": Wd4.reshape(H, 1),
        "bd1c": bdc(bd1), "bd2c": bdc(bd2), "bd3c": bdc(bd3),
        "bd4": bd4.reshape(1, 1),
    }

    in_maps = []
    rows_per_core = P * D // N_CORES  # 65536
    for r in range(N_CORES):
        sl = Wd1p[r * rows_per_core:(r + 1) * rows_per_core]
        # row = 512*i + 128*j + p  ->  [p, i, j, n] layout
        wd1s = np.ascontiguousarray(
            sl.reshape(S, 4, 128, H).transpose(2, 0, 1, 3).reshape(128, KCH * H))
        m = dict(common)
        m["ats"] = np.ascontiguousarray(AT[:, r * S:(r + 1) * S])
        m["wd1s"] = wd1s
        in_maps.append(m)
    return in_maps


def kernel(**inputs):
    from concourse.bass_utils import run_bass_kernel_spmd
    in_maps = make_in_maps(inputs)
    nc = _get_nc()
    res = run_bass_kernel_spmd(nc, in_maps, core_ids=list(range(N_CORES)))
    return np.asarray(res.results[0]["out"], np.float32).reshape(1)


# revision 6
# speedup vs baseline: 4964.0599x; 4964.0599x over previous
"""DeepHamCritic (3x GCNConv + dense head) on 8 trn2 NeuronCores.

Strategy:
  - Host: build dense normalized adjacency A [1000,1000] from edge_index
    (self-loops + deg^-1/2 symmetric norm), pad nodes 1000 -> 1024.
  - GCN layers sharded by destination node (128 padded nodes per core):
    aggregation = dense matmul against the local A^T column slice,
    AllGather of node features between layers.
  - Dense head: Wd1 [512000,256] row-sharded (64Ki rows/core, fp16),
    streamed through SBUF slabs into a PE matvec accumulated in PSUM;
    partial [256] results AllGather'ed and summed on-chip; the tiny
    Wd2/Wd3/Wd4 layers are computed replicated on every core.
"""

import numpy as np

N_CORES = 8
N = 1000          # real nodes
P = 1024          # padded nodes
S = P // N_CORES  # nodes per core = 128
F = 128           # input features
D = 512           # GCN hidden
H = 256           # dense hidden
KCH = P * D // N_CORES // 128   # 512 f-chunks of 128 per core
SLAB_CH = 32                    # chunks per DMA slab
N_SLAB = KCH // SLAB_CH         # 16 slabs
SLAB_W = SLAB_CH * H            # 8192 fp16 cols per slab

_NC = None


def _build_nc(reps=1):
    import concourse.bacc as bacc
    import concourse.mybir as mybir
    import concourse.tile as tile

    f32 = mybir.dt.float32
    f16 = mybir.dt.float16
    RG = [list(range(N_CORES))]

    nc = bacc.Bacc("TRN2", target_bir_lowering=False, debug=False,
                   num_devices=N_CORES)

    # ---- I/O ----
    xk = nc.dram_tensor("xk", [P, F], f32, kind="ExternalInput")
    ats = nc.dram_tensor("ats", [P, S], f32, kind="ExternalInput")
    w1 = nc.dram_tensor("w1", [F, D], f32, kind="ExternalInput")
    w2 = nc.dram_tensor("w2", [D, D], f32, kind="ExternalInput")
    w3 = nc.dram_tensor("w3", [D, D], f32, kind="ExternalInput")
    b1bc = nc.dram_tensor("b1bc", [128, D], f32, kind="ExternalInput")
    b2bc = nc.dram_tensor("b2bc", [128, D], f32, kind="ExternalInput")
    b3bc = nc.dram_tensor("b3bc", [128, D], f32, kind="ExternalInput")
    ident = nc.dram_tensor("ident", [128, 128], f32, kind="ExternalInput")
    wd1s = nc.dram_tensor("wd1s", [128, KCH * H], f16, kind="ExternalInput")
    wd2 = nc.dram_tensor("wd2", [H, H], f32, kind="ExternalInput")
    wd3 = nc.dram_tensor("wd3", [H, H], f32, kind="ExternalInput")
    wd4 = nc.dram_tensor("wd4", [H, 1], f32, kind="ExternalInput")
    bd1c = nc.dram_tensor("bd1c", [128, 2], f32, kind="ExternalInput")
    bd2c = nc.dram_tensor("bd2c", [128, 2], f32, kind="ExternalInput")
    bd3c = nc.dram_tensor("bd3c", [128, 2], f32, kind="ExternalInput")
    bd4 = nc.dram_tensor("bd4", [1, 1], f32, kind="ExternalInput")
    out = nc.dram_tensor("out", [1, 1], f32, kind="ExternalOutput")

    Tanh = mybir.ActivationFunctionType.Tanh
    Bypass = mybir.AluOpType.bypass

    with tile.TileContext(nc) as tc:
        with (
            tc.tile_pool(name="wslab", bufs=7) as wpool,
            tc.tile_pool(name="const", bufs=1) as cp,
            tc.tile_pool(name="hfull", bufs=8) as hp,
            tc.tile_pool(name="work", bufs=2) as wk,
            tc.tile_pool(name="psum", bufs=2, space="PSUM") as pp,
            tc.tile_pool(name="psacc", bufs=1, space="PSUM") as pacc,
            tc.tile_pool(name="dram", bufs=1, space="DRAM") as dp,
        ):
          for _rep in range(reps):
            # ---- load constants ----
            xk_t = []
            ats_t = []
            for q in range(8):
                t = cp.tile([128, F], f32, tag=f"xk{q}")
                nc.sync.dma_start(t[:], xk[q * 128:(q + 1) * 128, :])
                xk_t.append(t)
                a = cp.tile([128, S], f32, tag=f"ats{q}")
                nc.sync.dma_start(a[:], ats[q * 128:(q + 1) * 128, :])
                ats_t.append(a)
            w1_t = cp.tile([F, D], f32, tag="w1")
            nc.sync.dma_start(w1_t[:], w1[:])
            w2_t = []
            w3_t = []
            for m in range(4):
                t2 = cp.tile([128, D], f32, tag=f"w2{m}")
                nc.sync.dma_start(t2[:], w2[m * 128:(m + 1) * 128, :])
                w2_t.append(t2)
                t3 = cp.tile([128, D], f32, tag=f"w3{m}")
                nc.sync.dma_start(t3[:], w3[m * 128:(m + 1) * 128, :])
                w3_t.append(t3)
            b_t = []
            for nm, hndl in (("b1", b1bc), ("b2", b2bc), ("b3", b3bc)):
                t = cp.tile([128, D], f32, tag=nm)
                nc.sync.dma_start(t[:], hndl[:])
                b_t.append(t)
            id_t = cp.tile([128, 128], f32, tag="ident")
            nc.sync.dma_start(id_t[:], ident[:])
            wd2_t = []
            wd3_t = []
            for k in range(2):
                t2 = cp.tile([128, H], f32, tag=f"wd2{k}")
                nc.sync.dma_start(t2[:], wd2[k * 128:(k + 1) * 128, :])
                wd2_t.append(t2)
                t3 = cp.tile([128, H], f32, tag=f"wd3{k}")
                nc.sync.dma_start(t3[:], wd3[k * 128:(k + 1) * 128, :])
                wd3_t.append(t3)
            wd4_t = []
            for k in range(2):
                t4 = cp.tile([128, 1], f32, tag=f"wd4{k}")
                nc.sync.dma_start(t4[:], wd4[k * 128:(k + 1) * 128, :])
                wd4_t.append(t4)
            bd_t = []
            for nm, hndl in (("bd1", bd1c), ("bd2", bd2c), ("bd3", bd3c)):
                t = cp.tile([128, 2], f32, tag=nm)
                nc.sync.dma_start(t[:], hndl[:])
                bd_t.append(t)
            bd4_t = cp.tile([1, 1], f32, tag="bd4")
            nc.sync.dma_start(bd4_t[:], bd4[:])
            ones8 = cp.tile([8, 1], f32, tag="ones8")
            nc.vector.memset(ones8[:], 1.0)

            def leaky(dst_ap, ps_ap, bias_ap, mtag):
                t0 = wk.tile([128, 1], f32, tag=f"lk0{mtag}")
                nc.vector.tensor_add(t0[:], ps_ap, bias_ap)
                t1 = wk.tile([128, 1], f32, tag=f"lk1{mtag}")
                nc.vector.tensor_scalar_mul(t1[:], t0[:], 0.1)
                nc.vector.tensor_max(dst_ap, t0[:], t1[:])

            # ================= GCN =================
            # ---- layer 1 (local 128 dst nodes) ----
            ps_a1 = pp.tile([128, S], f32, tag="ps_sm")
            for k in range(8):
                nc.tensor.matmul(ps_a1[:], xk_t[k][:], ats_t[k][:],
                                 start=(k == 0), stop=(k == 7))
            a1 = wk.tile([128, S], f32, tag="a1")
            nc.vector.tensor_copy(a1[:], ps_a1[:])
            ps_h1 = pp.tile([128, D], f32, tag="ps_h")
            nc.tensor.matmul(ps_h1[:], a1[:], w1_t[:], start=True, stop=True)
            hb1 = wk.tile([128, D], f32, tag="hb")
            nc.vector.tensor_add(hb1[:], ps_h1[:], b_t[0][:])
            hs1 = wk.tile([128, D], f32, tag="hs")
            nc.scalar.activation(hs1[:], hb1[:], Tanh)

            # AllGather h1
            cc1i = dp.tile([128, D], f32, tag="cc1i")
            nc.sync.dma_start(cc1i[:], hs1[:])
            cc1o = dp.tile([P, D], f32, tag="cc1o", addr_space="Shared")
            nc.gpsimd.collective_compute(
                "AllGather", Bypass, replica_groups=RG,
                ins=[cc1i.opt()], outs=[cc1o.opt()])
            h1_t = []
            for q in range(8):
                t = hp.tile([128, D], f32, tag="hfull")
                nc.sync.dma_start(t[:], cc1o[q * 128:(q + 1) * 128, :])
                h1_t.append(t)

            def gcn_layer(h_in_t, w_chunks, b_tile, lname):
                # agg^T slices: [feat_chunk m partitions, S nodes]
                a2 = wk.tile([128, 4 * S], f32, tag=f"agg{lname}")
                for m in range(4):
                    ps = pp.tile([128, S], f32, tag="ps_sm")
                    for k in range(8):
                        nc.tensor.matmul(
                            ps[:], h_in_t[k][:, m * 128:(m + 1) * 128],
                            ats_t[k][:], start=(k == 0), stop=(k == 7))
                    nc.vector.tensor_copy(a2[:, m * S:(m + 1) * S], ps[:])
                ps_h = pp.tile([128, D], f32, tag="ps_h")
                for m in range(4):
                    nc.tensor.matmul(ps_h[:], a2[:, m * S:(m + 1) * S],
                                     w_chunks[m][:],
                                     start=(m == 0), stop=(m == 3))
                hb = wk.tile([128, D], f32, tag="hb")
                nc.vector.tensor_add(hb[:], ps_h[:], b_tile[:])
                hs = wk.tile([128, D], f32, tag="hs")
                nc.scalar.activation(hs[:], hb[:], Tanh)
                return hs

            # ---- layer 2 ----
            hs2 = gcn_layer(h1_t, w2_t, b_t[1], "l2")
            cc2i = dp.tile([128, D], f32, tag="cc2i")
            nc.sync.dma_start(cc2i[:], hs2[:])
            cc2o = dp.tile([P, D], f32, tag="cc2o", addr_space="Shared")
            nc.gpsimd.collective_compute(
                "AllGather", Bypass, replica_groups=RG,
                ins=[cc2i.opt()], outs=[cc2o.opt()])
            h2_t = []
            for q in range(8):
                t = hp.tile([128, D], f32, tag="hfull")
                nc.sync.dma_start(t[:], cc2o[q * 128:(q + 1) * 128, :])
                h2_t.append(t)

            # ---- layer 3 (local slice only; no gather) ----
            hs3 = gcn_layer(h2_t, w3_t, b_t[2], "l3")

            # ---- transpose local h3 [S,D] -> 4x [128, S] fp16 ----
            h3T = []
            for j in range(4):
                pst = pp.tile([128, S], f32, tag="ps_sm")
                nc.tensor.transpose(pst[:], hs3[:, j * 128:(j + 1) * 128],
                                    id_t[:])
                t16 = wk.tile([128, S], f16, tag=f"h3T{j}")
                nc.vector.tensor_copy(t16[:], pst[:])
                h3T.append(t16)

            # ================= dense head =================
            ps_y1 = pacc.tile([1, H], f32, tag="ps_y1")
            for g in range(N_SLAB):
                slab = wpool.tile([128, SLAB_W], f16, tag="slab")
                nc.sync.dma_start(slab[:], wd1s[:, g * SLAB_W:(g + 1) * SLAB_W])
                for t in range(SLAB_CH):
                    c = g * SLAB_CH + t
                    i, j = c // 4, c % 4
                    nc.tensor.matmul(
                        ps_y1[:], h3T[j][:, i:i + 1],
                        slab[:, t * H:(t + 1) * H],
                        start=(c == 0), stop=(c == KCH - 1))
            y1p = wk.tile([1, H], f32, tag="y1p")
            nc.vector.tensor_copy(y1p[:], ps_y1[:])

            # AllGather partials -> [8, H], then sum over partition dim
            ccyi = dp.tile([1, H], f32, tag="ccyi")
            nc.sync.dma_start(ccyi[:], y1p[:])
            ccyo = dp.tile([8, H], f32, tag="ccyo", addr_space="Shared")
            nc.gpsimd.collective_compute(
                "AllGather", Bypass, replica_groups=RG,
                ins=[ccyi.opt()], outs=[ccyo.opt()])
            y1g = wk.tile([8, H], f32, tag="y1g")
            nc.sync.dma_start(y1g[:], ccyo[:])

            y1c = wk.tile([128, 2], f32, tag="y1c")
            for m in range(2):
                ps = pp.tile([128, 1], f32, tag="ps_small")
                nc.tensor.matmul(ps[:], y1g[:, m * 128:(m + 1) * 128],
                                 ones8[:], start=True, stop=True)
                leaky(y1c[:, m:m + 1], ps[:], bd_t[0][:, m:m + 1], f"y1{m}")

            def dense(y_in, w_chunks, bias, oname):
                y_out = wk.tile([128, 2], f32, tag=oname)
                for m in range(2):
                    ps = pp.tile([128, 1], f32, tag="ps_small")
                    for k in range(2):
                        nc.tensor.matmul(
                            ps[:], w_chunks[k][:, m * 128:(m + 1) * 128],
                            y_in[:, k:k + 1], start=(k == 0), stop=(k == 1))
                    leaky(y_out[:, m:m + 1], ps[:], bias[:, m:m + 1],
                          f"{oname}{m}")
                return y_out

            y2c = dense(y1c, wd2_t, bd_t[1], "y2c")
            y3c = dense(y2c, wd3_t, bd_t[2], "y3c")

            ps_o = pp.tile([1, 1], f32, tag="ps_small")
            for k in range(2):
                nc.tensor.matmul(ps_o[:], wd4_t[k][:],
                                 y3c[:, k:k + 1], start=(k == 0), stop=(k == 1))
            out_sb = wk.tile([1, 1], f32, tag="out_sb")
            nc.vector.tensor_add(out_sb[:], ps_o[:], bd4_t[:])
            nc.sync.dma_start(out[:], out_sb[:])

    nc.compile()
    return nc


def _get_nc():
    global _NC
    if _NC is None:
        _NC = _build_nc()
    return _NC


def make_in_maps(inputs):
    """Host-side sharding / preprocessing. Returns per-core input dicts."""
    x = np.ascontiguousarray(np.asarray(inputs["x"], dtype=np.float32))
    ei = np.asarray(inputs["edge_index"])
    W1 = np.asarray(inputs["W1"], np.float32)
    W2 = np.asarray(inputs["W2"], np.float32)
    W3 = np.asarray(inputs["W3"], np.float32)
    b1 = np.asarray(inputs["b1"], np.float32)
    b2 = np.asarray(inputs["b2"], np.float32)
    b3 = np.asarray(inputs["b3"], np.float32)
    Wd1 = np.asarray(inputs["Wd1"], np.float32)
    Wd2 = np.asarray(inputs["Wd2"], np.float32)
    Wd3 = np.asarray(inputs["Wd3"], np.float32)
    Wd4 = np.asarray(inputs["Wd4"], np.float32)
    bd1 = np.asarray(inputs["bd1"], np.float32)
    bd2 = np.asarray(inputs["bd2"], np.float32)
    bd3 = np.asarray(inputs["bd3"], np.float32)
    bd4 = np.asarray(inputs["bd4"], np.float32)

    # normalized adjacency with self loops (GCNConv)
    src = ei[0].astype(np.int64)
    dst = ei[1].astype(np.int64)
    loop = np.arange(N, dtype=np.int64)
    s_all = np.concatenate([src, loop])
    d_all = np.concatenate([dst, loop])
    deg = np.bincount(d_all, minlength=N).astype(np.float32)
    dinv = np.where(deg > 0, 1.0 / np.sqrt(deg), 0.0).astype(np.float32)
    wnorm = dinv[s_all] * dinv[d_all]
    A = np.zeros((N, N), np.float32)
    np.add.at(A, (d_all, s_all), wnorm)
    AT = np.zeros((P, P), np.float32)
    AT[:N, :N] = A.T

    xk = np.zeros((P, F), np.float32)
    xk[:N] = x

    Wd1p = np.zeros((P * D, H), np.float16)
    Wd1p[:N * D] = Wd1.astype(np.float16)

    bb = lambda b: np.ascontiguousarray(np.broadcast_to(b[None, :], (128, b.shape[0])))
    bdc = lambda b: np.ascontiguousarray(b.reshape(2, 128).T)

    common = {
        "xk": xk,
        "w1": W1, "w2": W2, "w3": W3,
        "b1bc": bb(b1), "b2bc": bb(b2), "b3bc": bb(b3),
        "ident": np.eye(128, dtype=np.float32),
        "wd2": Wd2, "wd3": Wd3, "wd4": Wd4.reshape(H, 1),
        "bd1c": bdc(bd1), "bd2c": bdc(bd2), "bd3c": bdc(bd3),
        "bd4": bd4.reshape(1, 1),
    }

    in_maps = []
    rows_per_core = P * D // N_CORES  # 65536
    for r in range(N_CORES):
        sl = Wd1p[r * rows_per_core:(r + 1) * rows_per_core]
        # row = 512*i + 128*j + p  ->  [p, i, j, n] layout
        wd1s = np.ascontiguousarray(
            sl.reshape(S, 4, 128, H).transpose(2, 0, 1, 3).reshape(128, KCH * H))
        m = dict(common)
        m["ats"] = np.ascontiguousarray(AT[:, r * S:(r + 1) * S])
        m["wd1s"] = wd1s
        in_maps.append(m)
    return in_maps


def kernel(**inputs):
    from concourse.bass_utils import run_bass_kernel_spmd
    in_maps = make_in_maps(inputs)
    nc = _get_nc()
    res = run_bass_kernel_spmd(nc, in_maps, core_ids=list(range(N_CORES)))
    return np.asarray(res.results[0]["out"], np.float32).reshape(1)
